# revision 1
# baseline (speedup 1.0000x reference)
"""CrossAttention Trainium2 kernel — 8-core SPMD, no collectives.

Sharding: core c = (p, s) with p = c // 2 (head pair {2p, 2p+1}),
s = c % 2 (query shard: image rows [32s, 32s+32), tokens [2048s, +2048)).

Host composes the 3x3 convs with the q/k/v projections (both linear):
  k = conv(akv, Wk@kvw), v = conv(akv, Wv@kvw), q = SCALE*conv(x, Wq@qw)
so each core runs ONE 128-out-ch conv over full akv
([k_h0 pad32 | k_h1 pad32 | v_h0 | v_h1]) and one 64-out-ch conv over its
34-row x slice. Attention is flash-style: per 128-key block, two row-tiled
QK^T matmuls into a double-buffered 2-bank PSUM group (strips alternate by
key-block parity for 4-way PE tiling), one ACT exp over [128, 1024], AV
with a fused ones-column denominator. The scrambled output reshape
(o[h,n,d] -> rows n2=512h+n//8, ch (n%8)*32+d) is realized with
shift-matmuls against host-built shifted identities.

Per-chunk tiles keep dependencies fine-grained so attention overlaps the
conv. ACT runs exp only; all copies are on DVE. Matmul dtypes: fp32r for
convs (DMA-fed), bf16 for the attention path.
"""

import numpy as np

import concourse.bass as bass
import concourse.mybir as mybir
import concourse.tile as tile
from concourse import bacc, bass_utils

DIM = 256
HEADS = 8
HEAD_DIM = 32          # v head dim
QK_DIM = 16            # q/k head dim
SCALE = HEAD_DIM ** -0.5
H = W = 64
N = H * W              # 4096 tokens
N_CORES = 8
NQ = N // 2            # queries per core (2048)
KB = 128               # key block size
N_KB = N // KB         # 32 key blocks
QC = 512               # query chunk (matmul N)
N_QC = NQ // QC        # 4 query chunks

F32 = mybir.dt.float32
F32R = mybir.dt.float32r
BF16 = mybir.dt.bfloat16
AF = mybir.ActivationFunctionType

_CACHE = {}


def build_nc(n_reps: int = 1):
    """Build + compile the SPMD Bass program (same NEFF on all 8 cores)."""
    key = ("nc", n_reps)
    if key in _CACHE:
        return _CACHE[key]
    nc = bacc.Bacc("TRN2", target_bir_lowering=False, debug=False,
                   num_devices=N_CORES)

    d = {}
    d["akv"] = nc.dram_tensor("akv", [DIM, 66 * 66], F32R, kind="ExternalInput").ap()
    d["xs"] = nc.dram_tensor("xs", [DIM, 34 * 66], F32R, kind="ExternalInput").ap()
    d["wkv"] = nc.dram_tensor("wkv", [DIM, 9, 128], F32R, kind="ExternalInput").ap()
    d["wq"] = nc.dram_tensor("wq", [DIM, 9, 64], F32R, kind="ExternalInput").ap()
    d["bkv"] = nc.dram_tensor("bkv", [1, 128], F32R, kind="ExternalInput").ap()
    d["bq"] = nc.dram_tensor("bq", [1, 64], F32R, kind="ExternalInput").ap()
    d["ones512"] = nc.dram_tensor("ones512", [1, 512], F32R, kind="ExternalInput").ap()
    d["ones32"] = nc.dram_tensor("ones32", [1, 32], F32R, kind="ExternalInput").ap()
    d["ident64"] = nc.dram_tensor("ident64", [64, 64], F32, kind="ExternalInput").ap()
    d["identsh"] = nc.dram_tensor("identsh", [32, 4, 128], BF16, kind="ExternalInput").ap()
    d["wpt"] = nc.dram_tensor("wpt", [DIM, DIM], BF16, kind="ExternalInput").ap()
    d["bp"] = nc.dram_tensor("bp", [128, 2], F32, kind="ExternalInput").ap()
    out_d = nc.dram_tensor("out", [DIM, 512], F32, kind="ExternalOutput").ap()

    with tile.TileContext(nc) as tc:
        if n_reps == 1:
            _emit(nc, tc, d, out_d)
        else:
            with tc.For_i(0, n_reps, 1):
                _emit(nc, tc, d, out_d)

    nc.compile()
    _CACHE[key] = nc
    return nc


def _emit(nc, tc, d, out_d):
    import contextlib
    ctx = contextlib.ExitStack()
    with ctx:
        consts = ctx.enter_context(tc.tile_pool(name="consts", bufs=1))
        big = ctx.enter_context(tc.tile_pool(name="big", bufs=1))
        epool = ctx.enter_context(tc.tile_pool(name="epool", bufs=3))
        small = ctx.enter_context(tc.tile_pool(name="small", bufs=4))
        qk_ps = ctx.enter_context(tc.tile_pool(name="qk_ps", bufs=2, space="PSUM"))
        av_ps = ctx.enter_context(tc.tile_pool(name="av_ps", bufs=2, space="PSUM"))
        aux_ps = ctx.enter_context(tc.tile_pool(name="aux_ps", bufs=2, space="PSUM"))

        # ---- constants / weights -------------------------------------------
        wkv_sb = consts.tile([128, 2, 9, 128], F32R, tag="wkv_sb")
        nc.sync.dma_start(out=wkv_sb,
                          in_=d["wkv"].rearrange("(b p) t o -> p b t o", p=128))
        wq_sb = consts.tile([128, 2, 9, 64], F32R, tag="wq_sb")
        nc.sync.dma_start(out=wq_sb,
                          in_=d["wq"].rearrange("(b p) t o -> p b t o", p=128))
        bkv_sb = consts.tile([1, 128], F32R, tag="bkv_sb")
        nc.sync.dma_start(out=bkv_sb, in_=d["bkv"])
        bq_sb = consts.tile([1, 64], F32R, tag="bq_sb")
        nc.sync.dma_start(out=bq_sb, in_=d["bq"])
        ones512 = consts.tile([1, 512], F32R, tag="ones512")
        nc.sync.dma_start(out=ones512, in_=d["ones512"])
        ones32 = consts.tile([1, 32], F32R, tag="ones32")
        nc.sync.dma_start(out=ones32, in_=d["ones32"])
        ident64 = consts.tile([128, 64], F32, tag="ident64")
        nc.sync.dma_start(out=ident64[64:128, :], in_=d["ident64"])
        identsh = consts.tile([32, 4, 128], BF16, tag="identsh")
        nc.sync.dma_start(out=identsh, in_=d["identsh"])
        wpt_sb = consts.tile([128, 2, 256], BF16, tag="wpt_sb")
        nc.sync.dma_start(out=wpt_sb,
                          in_=d["wpt"].rearrange("(b p) o -> p b o", p=128))
        bp_sb = consts.tile([128, 2], F32, tag="bp_sb")
        nc.sync.dma_start(out=bp_sb, in_=d["bp"])

        # ---- inputs ---------------------------------------------------------
        akv_sb = big.tile([128, 2, 66, 66], F32R, tag="akv_sb")
        nc.sync.dma_start(out=akv_sb,
                          in_=d["akv"].rearrange("(b p) (y x) -> p b y x", p=128, y=66))
        xs_sb = big.tile([128, 2, 34, 66], F32R, tag="xs_sb")
        nc.sync.dma_start(out=xs_sb,
                          in_=d["xs"].rearrange("(b p) (y x) -> p b y x", p=128, y=34))

        # ---- persistent intermediates (per-chunk tiles for fine deps) -------
        kT = [big.tile([128, 512], BF16, tag=f"kT{c}", name=f"kT{c}")
              for c in range(8)]
        qTt = [big.tile([128, 512], BF16, tag=f"qT{c}", name=f"qT{c}")
               for c in range(4)]
        vT = [big.tile([128, 512], F32, tag=f"vT{c}", name=f"vT{c}")
              for c in range(8)]
        Vt = [big.tile([128, 4, 66], BF16, tag=f"V{c}", name=f"V{c}")
              for c in range(8)]
        o_nrm = big.tile([32, 2, NQ], BF16, tag="o_nrm")
        rowsT = big.tile([128, 2, 512], BF16, tag="rowsT")

        for c in range(8):
            nc.vector.memset(Vt[c][:, :, 32:33], 1.0)
            nc.vector.memset(Vt[c][:, :, 65:66], 1.0)

        # ---- kv conv: akv -> [k_h0pad32 | k_h1pad32 | v_h0 | v_h1] ----------
        for c in range(8):                                  # 8 pixel chunks of 512
            ps = aux_ps.tile([128, 512], F32, tag="aux", name=f"cvkv{c}")
            first = True
            for cib in range(2):
                for ky in range(3):
                    for kx in range(3):
                        rhs = akv_sb[:, cib, c * 8 + ky: c * 8 + ky + 8, kx: kx + 64]
                        nc.tensor.matmul(
                            ps, wkv_sb[:, cib, ky * 3 + kx, :], rhs,
                            start=first, stop=False, skip_group_check=True)
                        first = False
            nc.tensor.matmul(ps, bkv_sb, ones512, start=False, stop=True,
                             skip_group_check=True)
            # rows 0:64 -> kT strips (bf16); rows 64:128 -> vT (f32)
            nc.vector.tensor_copy(kT[c][0:64, :], ps[0:64, :])
            nc.sync.dma_start(out=kT[c][64:128, :], in_=kT[c][0:64, :])
            nc.vector.tensor_copy(vT[c][64:128, :], ps[64:128, :])
            # transpose v for the 4 key blocks of this chunk
            for j in range(4):
                tp = aux_ps.tile([128, 512], F32, tag="aux", name=f"tp{c}_{j}")
                nc.tensor.transpose(tp[:, 0:64],
                                    vT[c][64:128, j * 128:(j + 1) * 128],
                                    ident64[64:128, :])
                nc.vector.tensor_copy(out=Vt[c][:, j, 0:32], in_=tp[:, 0:32])
                nc.vector.tensor_copy(out=Vt[c][:, j, 33:65], in_=tp[:, 32:64])

        # ---- q conv: xs -> [q_h0 pad32 | q_h1 pad32] ------------------------
        for c in range(4):                                  # 4 chunks of 512
            ps = aux_ps.tile([128, 512], F32, tag="aux", name=f"cvq{c}")
            first = True
            for cib in range(2):
                for ky in range(3):
                    for kx in range(3):
                        rhs = xs_sb[:, cib, c * 8 + ky: c * 8 + ky + 8, kx: kx + 64]
                        nc.tensor.matmul(
                            ps[0:64, :], wq_sb[:, cib, ky * 3 + kx, :], rhs,
                            start=first, stop=False, skip_group_check=True)
                        first = False
            nc.tensor.matmul(ps[0:64, :], bq_sb, ones512, start=False, stop=True,
                             skip_group_check=True)
            nc.vector.tensor_copy(qTt[c][0:64, :], ps[0:64, :])
            nc.sync.dma_start(out=qTt[c][64:128, :], in_=qTt[c][0:64, :])

        # ---- attention ------------------------------------------------------
        for qc in range(N_QC):
            acc = [av_ps.tile([33, 512], F32, tag="av", name=f"acc{qc}_{i}")
                   for i in range(2)]
            for kb in range(N_KB):
                sp = 2 * (kb % 2)                           # strip pair 0/2
                cc, j = kb // 4, kb % 4
                lg = qk_ps.tile([128, 2, 512], F32, tag="qk", name=f"lg{qc}_{kb}")
                for hl in range(2):
                    i = sp + hl
                    nc.tensor.matmul(
                        lg[:, hl, :],
                        kT[cc][32 * i:32 * i + 32, j * 128:(j + 1) * 128],
                        qTt[qc][32 * i:32 * i + 32, :],
                        start=True, stop=True, skip_group_check=True,
                        tile_position=(32 * i, 0))
                eg = epool.tile([128, 2, 512], BF16, tag="eg", name=f"eg{qc}_{kb}")
                nc.scalar.activation(eg, lg, AF.Exp)
                for hl in range(2):
                    nc.tensor.matmul(
                        acc[hl], Vt[cc][:, j, 33 * hl: 33 * hl + 33],
                        eg[:, hl, :],
                        start=(kb == 0), stop=(kb == N_KB - 1),
                        skip_group_check=True)
            qsl = slice(qc * 512, (qc + 1) * 512)
            for hl in range(2):
                r = small.tile([1, 512], F32R, tag="recip", name=f"r{qc}_{hl}")
                with nc.allow_low_precision(reason="fp32r recip is ~19-bit"):
                    nc.vector.reciprocal(r, acc[hl][32:33, :])
                rb = aux_ps.tile([128, 512], F32, tag="aux", name=f"rb{qc}_{hl}")
                nc.tensor.matmul(rb[0:32, :], ones32, r, start=True, stop=True,
                                 skip_group_check=True)
                rbs = small.tile([32, 512], F32, tag="rbs", name=f"rbs{qc}_{hl}")
                nc.vector.tensor_copy(rbs, rb[0:32, :])
                nc.vector.tensor_mul(o_nrm[:, hl, qsl],
                                     acc[hl][0:32, :], rbs)

        # ---- scramble shuffle (shift-matmuls) + output projection -----------
        o_j = o_nrm.rearrange("p h (m j) -> p h j m", j=8)   # [32, 2, 8, 256]
        for icb in range(2):
            rp = aux_ps.tile([128, 512], F32, tag="aux", name=f"rp{icb}")
            for hl in range(2):
                for jm in range(4):
                    j = 4 * icb + jm
                    nc.tensor.matmul(
                        rp[:, 256 * hl:256 * hl + 256],
                        identsh[:, jm, :], o_j[:, hl, j, :],
                        start=(jm == 0), stop=(jm == 3), skip_group_check=True)
            nc.vector.tensor_copy(rowsT[:, icb, :], rp)
        for ocb in range(2):
            po = aux_ps.tile([128, 512], F32, tag="aux", name=f"po{ocb}")
            for icb in range(2):
                nc.tensor.matmul(po, wpt_sb[:, icb, 128 * ocb:128 * ocb + 128],
                                 rowsT[:, icb, :], start=(icb == 0),
                                 stop=(icb == 1), skip_group_check=True)
            ob = small.tile([128, 512], F32, tag="outsb", name=f"ob{ocb}")
            nc.vector.tensor_scalar_add(ob, po, bp_sb[:, ocb:ocb + 1])
            nc.sync.dma_start(out=out_d[128 * ocb:128 * ocb + 128, :], in_=ob)


# --------------------------------------------------------------------------
# host side
# --------------------------------------------------------------------------

def host_prep(x, attn_kv, qw, qb, kvw, kvb, Wq, bq, Wk, bk, Wv, bv, Wp, bp):
    import ml_dtypes
    f = np.float32
    x = np.asarray(x, f)[0]          # [256, 64, 64]
    akv = np.asarray(attn_kv, f)[0]
    Wqc = np.einsum("jc,ciyx->jiyx", np.asarray(Wq, f), np.asarray(qw, f)) * SCALE
    Wkc = np.einsum("jc,ciyx->jiyx", np.asarray(Wk, f), np.asarray(kvw, f))
    Wvc = np.einsum("jc,ciyx->jiyx", np.asarray(Wv, f), np.asarray(kvw, f))
    bqc = (np.asarray(Wq, f) @ np.asarray(qb, f) + np.asarray(bq, f)) * SCALE
    bkc = np.asarray(Wk, f) @ np.asarray(kvb, f) + np.asarray(bk, f)
    bvc = np.asarray(Wv, f) @ np.asarray(kvb, f) + np.asarray(bv, f)

    akv_p = np.zeros((DIM, 66, 66), f)
    akv_p[:, 1:65, 1:65] = akv
    x_p = np.zeros((DIM, 66, 66), f)
    x_p[:, 1:65, 1:65] = x

    per_pair = []
    for p in range(4):
        wkv = np.zeros((128, DIM, 3, 3), f)
        bkv = np.zeros((128,), f)
        wq_ = np.zeros((64, DIM, 3, 3), f)
        bq_ = np.zeros((64,), f)
        for hl in range(2):
            h = 2 * p + hl
            wkv[32 * hl:32 * hl + QK_DIM] = Wkc[QK_DIM * h:QK_DIM * (h + 1)]
            bkv[32 * hl:32 * hl + QK_DIM] = bkc[QK_DIM * h:QK_DIM * (h + 1)]
            wkv[64 + 32 * hl:64 + 32 * (hl + 1)] = Wvc[HEAD_DIM * h:HEAD_DIM * (h + 1)]
            bkv[64 + 32 * hl:64 + 32 * (hl + 1)] = bvc[HEAD_DIM * h:HEAD_DIM * (h + 1)]
            wq_[32 * hl:32 * hl + QK_DIM] = Wqc[QK_DIM * h:QK_DIM * (h + 1)]
            bq_[32 * hl:32 * hl + QK_DIM] = bqc[QK_DIM * h:QK_DIM * (h + 1)]
        per_pair.append((
            np.ascontiguousarray(wkv.transpose(1, 2, 3, 0).reshape(DIM, 9, 128)),
            bkv.reshape(1, 128),
            np.ascontiguousarray(wq_.transpose(1, 2, 3, 0).reshape(DIM, 9, 64)),
            bq_.reshape(1, 64),
        ))

    identsh = np.zeros((32, 4, 128), ml_dtypes.bfloat16)
    for jm in range(4):
        for dd in range(32):
            identsh[dd, jm, 32 * jm + dd] = 1.0
    wpt = np.ascontiguousarray(np.asarray(Wp, f).T).astype(ml_dtypes.bfloat16)
    bp_a = np.ascontiguousarray(np.asarray(bp, f).reshape(2, 128).T)

    in_maps = []
    for c in range(N_CORES):
        p, s = c // 2, c % 2
        wkv_h, bkv_h, wq_h, bq_h = per_pair[p]
        in_maps.append({
            "akv": akv_p.reshape(DIM, -1),
            "xs": np.ascontiguousarray(x_p[:, 32 * s:32 * s + 34, :]).reshape(DIM, -1),
            "wkv": wkv_h, "bkv": bkv_h, "wq": wq_h, "bq": bq_h,
            "ones512": np.ones((1, 512), f), "ones32": np.ones((1, 32), f),
            "ident64": np.eye(64, dtype=f), "identsh": identsh,
            "wpt": wpt, "bp": bp_a,
        })
    return in_maps


def gather(results):
    full = np.empty((DIM, N), np.float32)
    for c in range(N_CORES):
        p, s = c // 2, c % 2
        dev = results[c]["out"]
        for hl in range(2):
            h = 2 * p + hl
            full[:, 512 * h + 256 * s: 512 * h + 256 * s + 256] = \
                dev[:, 256 * hl:256 * hl + 256]
    return full.reshape(1, DIM, H, W)


def kernel(x, attn_kv, qw, qb, kvw, kvb, Wq, bq, Wk, bk, Wv, bv, Wp, bp):
    nc = build_nc()
    in_maps = host_prep(x, attn_kv, qw, qb, kvw, kvb, Wq, bq, Wk, bk, Wv, bv,
                        Wp, bp)
    res = bass_utils.run_bass_kernel_spmd(nc, in_maps,
                                          core_ids=list(range(N_CORES)),
                                          trace=False)
    return gather(res.results).astype(np.float32)



# revision 22
# speedup vs baseline: 1.0564x; 1.0564x over previous
"""CrossAttention Trainium2 kernel — 8-core SPMD, no collectives.

Sharding: core c = (p, s) with p = c // 2 (head pair {2p, 2p+1}),
s = c % 2 (query shard: image rows [32s, 32s+32), tokens [2048s, +2048)).

Host composes the 3x3 convs with the q/k/v projections (both linear):
  k = conv(akv, Wk@kvw), v = conv(akv, Wv@kvw), q = SCALE*conv(x, Wq@qw)
so each core runs ONE 128-out-ch conv over full akv
([k_h0 pad32 | k_h1 pad32 | v_h0 | v_h1]) and one 64-out-ch conv over its
34-row x slice.  Everything on the PE runs in bf16 (fp32 matmuls measure
~3x slower and fp32 LDWEIGHTS doesn't pipeline); conv accumulation stays
fp32 in PSUM and biases are folded into the PSUM->SBUF copies.

Attention processes KEY-BLOCK PAIRS: four row-tiled QK^T matmuls (strips
0-3 = kb even h0/h1, kb odd h0/h1, all four PE row groups -> one
concurrent PE window) fill two double-buffered 2-bank PSUM tiles, two
ACT exps cover [128, 1024] each (ACT is the roofline: 16.8M exps/core),
then two 4-way col-tiled AV quads (numerators for both heads at col
groups 0/1, ones-column softmax denominators at 2/3) accumulate into
ONE PSUM bank per query chunk.  AV emission lags QK/exp by one pair so
PE never head-blocks on an exp.  Conv work is sliced into micro-ops and
pumped between attention steps (phase 1 sweeps query chunks {0,1}
behind the conv stream; phase 2 sweeps qc2 then qc3 sequentially so
normalization and the scramble overlap the remaining sweep).

Softmax normalization computes 1/denominator as exp(-ln d) on the ACT
engine (the exact DVE reciprocal measures 3.4us per row) and broadcasts
it across 32 partitions with a ones-column matmul; build_nc() empties
the exp-only/ln-only activation-table sets so walrus settles on the one
table holding both (otherwise it thrashes 16 table loads = 20us).  The
scrambled output reshape (o[h,n,d] -> rows n2=512h+n//8, ch (n%8)*32+d)
is realized with shift-matmuls against host-built shifted identities,
emitted in query sub-ranges as each chunk's normalization lands.
"""

import numpy as np

import concourse.bass as bass
import concourse.mybir as mybir
import concourse.tile as tile
from concourse import bacc, bass_utils

DIM = 256
HEADS = 8
HEAD_DIM = 32          # v head dim
QK_DIM = 16            # q/k head dim
SCALE = HEAD_DIM ** -0.5
H = W = 64
N = H * W              # 4096 tokens
N_CORES = 8
NQ = N // 2            # queries per core (2048)
KB = 128               # key block size
N_KB = N // KB         # 32 key blocks
QC = 512               # query chunk (matmul N)
N_QC = NQ // QC        # 4 query chunks

F32 = mybir.dt.float32
F32R = mybir.dt.float32r
BF16 = mybir.dt.bfloat16
AF = mybir.ActivationFunctionType

_CACHE = {}


def build_nc(n_reps: int = 1):
    """Build + compile the SPMD Bass program (same NEFF on all 8 cores)."""
    key = ("nc", n_reps)
    if key in _CACHE:
        return _CACHE[key]
    nc = bacc.Bacc("TRN2", target_bir_lowering=False, debug=False,
                   num_devices=N_CORES)

    # Steer the activation-table pass to the one set holding BOTH exp and
    # ln: empty the exp-only / ln-only sets in the (cached) table dict so
    # the fixpoint can't bounce between them (16 ACT_TABLE_LOADs = 20us of
    # ACT otherwise).  Positions are preserved, so act_func_set_ids stay
    # valid; this kernel only uses Exp and Ln.
    from concourse.hw_specs import get_activation_tables
    tabs = get_activation_tables(nc.m.arch)
    if "natural_log_exp_and_others" in tabs:
        for name in ("exp_and_others", "natural_log", "exp_and_friends"):
            if name in tabs:
                tabs[name].clear()

    d = {}
    d["akv"] = nc.dram_tensor("akv", [DIM, 66 * 66], BF16, kind="ExternalInput").ap()
    d["xs"] = nc.dram_tensor("xs", [DIM, 34 * 66], BF16, kind="ExternalInput").ap()
    d["wkv"] = nc.dram_tensor("wkv", [DIM, 9, 128], BF16, kind="ExternalInput").ap()
    d["wq"] = nc.dram_tensor("wq", [DIM, 9, 64], BF16, kind="ExternalInput").ap()
    d["bkv"] = nc.dram_tensor("bkv", [128, 1], F32, kind="ExternalInput").ap()
    d["bq"] = nc.dram_tensor("bq", [64, 1], F32, kind="ExternalInput").ap()
    d["ones32"] = nc.dram_tensor("ones32", [1, 32], F32R, kind="ExternalInput").ap()
    d["ident64"] = nc.dram_tensor("ident64", [64, 64], BF16, kind="ExternalInput").ap()
    d["identsh"] = nc.dram_tensor("identsh", [32, 4, 128], BF16, kind="ExternalInput").ap()
    d["wpt"] = nc.dram_tensor("wpt", [DIM, DIM], BF16, kind="ExternalInput").ap()
    d["bp"] = nc.dram_tensor("bp", [128, 2], F32, kind="ExternalInput").ap()
    out_d = nc.dram_tensor("out", [DIM, 512], F32, kind="ExternalOutput").ap()

    with tile.TileContext(nc) as tc:
        if n_reps == 1:
            _emit(nc, tc, d, out_d)
        else:
            with tc.For_i(0, n_reps, 1):
                _emit(nc, tc, d, out_d)

    nc.compile()
    _CACHE[key] = nc
    return nc


def _emit(nc, tc, d, out_d):
    import contextlib
    ctx = contextlib.ExitStack()
    with ctx:
        consts = ctx.enter_context(tc.tile_pool(name="consts", bufs=1))
        big = ctx.enter_context(tc.tile_pool(name="big", bufs=1))
        vpool = ctx.enter_context(tc.tile_pool(name="vpool", bufs=2))
        epool = ctx.enter_context(tc.tile_pool(name="epool", bufs=4))
        small = ctx.enter_context(tc.tile_pool(name="small", bufs=4))
        qk_ps = ctx.enter_context(tc.tile_pool(name="qk_ps", bufs=2, space="PSUM"))
        av_ps = ctx.enter_context(tc.tile_pool(name="av_ps", bufs=2, space="PSUM"))
        aux_ps = ctx.enter_context(tc.tile_pool(name="aux_ps", bufs=1, space="PSUM"))
        tp_ps = ctx.enter_context(tc.tile_pool(name="tp_ps", bufs=1, space="PSUM"))

        # ---- constants / weights (DMA order = dependency order) ------------
        wq_sb = consts.tile([128, 2, 9, 64], BF16, tag="wq_sb")
        nc.sync.dma_start(out=wq_sb,
                          in_=d["wq"].rearrange("(b p) t o -> p b t o", p=128))
        bq_sb = consts.tile([64, 1], F32, tag="bq_sb")
        nc.sync.dma_start(out=bq_sb, in_=d["bq"])

        # warm the ln+exp activation table while PE does the first convs
        # (Log first so walrus settles on natural_log_exp_and_others once)
        sc1 = small.tile([1, 32], F32, tag="w1", name="actw1")
        nc.vector.memset(sc1, 1.0)
        sc2 = small.tile([1, 32], F32, tag="w2", name="actw2")
        nc.scalar.activation(sc2, sc1, AF.Ln)
        sc3 = small.tile([1, 32], F32, tag="w3", name="actw3")
        nc.scalar.activation(sc3, sc2, AF.Exp)

        akv_sb = big.tile([128, 2, 66, 66], BF16, tag="akv_sb")
        xs_sb = big.tile([128, 2, 34, 66], BF16, tag="xs_sb")
        akv_r = d["akv"].rearrange("(b p) (y x) -> p b y x", p=128, y=66)
        xs_r = d["xs"].rearrange("(b p) (y x) -> p b y x", p=128, y=34)
        xs_rows = [(0, 10), (10, 18), (18, 26), (26, 34)]
        akv_rows = [(0, 10)] + [(8 * c + 2, 8 * c + 10) for c in range(1, 8)]
        nc.sync.dma_start(out=xs_sb[:, :, 0:10, :], in_=xs_r[:, :, 0:10, :])

        wkv_sb = consts.tile([128, 2, 9, 128], BF16, tag="wkv_sb")
        nc.sync.dma_start(out=wkv_sb,
                          in_=d["wkv"].rearrange("(b p) t o -> p b t o", p=128))
        bkv_sb = consts.tile([128, 1], F32, tag="bkv_sb")
        nc.sync.dma_start(out=bkv_sb, in_=d["bkv"])
        ident64 = consts.tile([128, 64], BF16, tag="ident64")
        nc.sync.dma_start(out=ident64[64:128, :], in_=d["ident64"])
        nc.sync.dma_start(out=akv_sb[:, :, 0:10, :], in_=akv_r[:, :, 0:10, :])

        for i in range(1, 4):
            r0, r1 = xs_rows[i]
            nc.sync.dma_start(out=xs_sb[:, :, r0:r1, :], in_=xs_r[:, :, r0:r1, :])
            r0, r1 = akv_rows[i]
            nc.sync.dma_start(out=akv_sb[:, :, r0:r1, :], in_=akv_r[:, :, r0:r1, :])
        for i in range(4, 8):
            r0, r1 = akv_rows[i]
            nc.sync.dma_start(out=akv_sb[:, :, r0:r1, :], in_=akv_r[:, :, r0:r1, :])

        ones32 = consts.tile([1, 32], F32R, tag="ones32")
        nc.sync.dma_start(out=ones32, in_=d["ones32"])
        identsh = consts.tile([32, 4, 128], BF16, tag="identsh")
        nc.sync.dma_start(out=identsh, in_=d["identsh"])
        wpt_sb = consts.tile([128, 2, 256], BF16, tag="wpt_sb")
        nc.sync.dma_start(out=wpt_sb,
                          in_=d["wpt"].rearrange("(b p) o -> p b o", p=128))
        bp_sb = consts.tile([128, 2], F32, tag="bp_sb")
        nc.sync.dma_start(out=bp_sb, in_=d["bp"])

        # ---- persistent intermediates --------------------------------------
        kT = [big.tile([128, 512], BF16, tag=f"kT{c}", name=f"kT{c}")
              for c in range(8)]
        qTt = [big.tile([128, 512], BF16, tag=f"qT{c}", name=f"qT{c}")
               for c in range(4)]
        Vt = [big.tile([128, 4, 65], BF16, tag=f"V{c}", name=f"V{c}")
              for c in range(8)]
        o_nrm = big.tile([32, 2, NQ], BF16, tag="o_nrm")
        rowsT = big.tile([128, 2, 512], BF16, tag="rowsT")

        for c in range(8):
            nc.vector.memset(Vt[c][:, :, 64:65], 1.0)

        # ---- conv micro-op queue (sliced between attention steps) ----------
        from collections import deque
        conv_q = deque()          # (batch_id, closure)

        def kvconv_ops(c):
            st = {}

            def tap(cib, ky, kx, first, last):
                def f():
                    if first:
                        st["ps"] = aux_ps.tile([128, 512], F32, tag="aux",
                                               name=f"cvkv{c}")
                    rhs = akv_sb[:, cib, c * 8 + ky: c * 8 + ky + 8, kx: kx + 64]
                    nc.tensor.matmul(
                        st["ps"], wkv_sb[:, cib, ky * 3 + kx, :], rhs,
                        start=first, stop=last, skip_group_check=True)
                return f

            ops = []
            idx = 0
            for cib in range(2):
                for ky in range(3):
                    for kx in range(3):
                        ops.append(tap(cib, ky, kx, idx == 0, idx == 17))
                        idx += 1

            def copies():
                ps = st["ps"]
                nc.vector.tensor_scalar_add(kT[c][0:64, :], ps[0:64, :],
                                            bkv_sb[0:64, :])
                nc.gpsimd.dma_start(out=kT[c][64:128, :], in_=kT[c][0:64, :])
                st["vs"] = vpool.tile([128, 512], BF16, tag="vstage",
                                      name=f"vs{c}")
                nc.vector.tensor_scalar_add(st["vs"][64:128, :], ps[64:128, :],
                                            bkv_sb[64:128, :])
            ops.append(copies)

            def tr(j):
                def f():
                    tp = tp_ps.tile([128, 64], BF16, tag="tp", name=f"tp{c}_{j}")
                    nc.tensor.transpose(tp,
                                        st["vs"][64:128, j * 128:(j + 1) * 128],
                                        ident64[64:128, :])
                    nc.vector.tensor_copy(out=Vt[c][:, j, 0:64], in_=tp)
                return f
            ops += [tr(j) for j in range(4)]
            return ops

        def qconv_ops(c):
            st = {}

            def tap(cib, ky, kx, first, last):
                def f():
                    if first:
                        st["ps"] = aux_ps.tile([128, 512], F32, tag="aux",
                                               name=f"cvq{c}")
                    rhs = xs_sb[:, cib, c * 8 + ky: c * 8 + ky + 8, kx: kx + 64]
                    nc.tensor.matmul(
                        st["ps"][0:64, :], wq_sb[:, cib, ky * 3 + kx, :], rhs,
                        start=first, stop=last, skip_group_check=True)
                return f

            ops = []
            idx = 0
            for cib in range(2):
                for ky in range(3):
                    for kx in range(3):
                        ops.append(tap(cib, ky, kx, idx == 0, idx == 17))
                        idx += 1

            def copies():
                nc.vector.tensor_scalar_add(qTt[c][0:64, :], st["ps"][0:64, :],
                                            bq_sb)
                nc.gpsimd.dma_start(out=qTt[c][64:128, :], in_=qTt[c][0:64, :])
            ops.append(copies)
            return ops

        def queue_conv(batch, ops):
            conv_q.extend((batch, f) for f in ops)

        def pump(n):
            for _ in range(n):
                if not conv_q:
                    return
                conv_q.popleft()[1]()

        def drain(batch):
            while conv_q and conv_q[0][0] <= batch:
                conv_q.popleft()[1]()

        # ---- attention stream: kb-PAIR steps, AV lags by one pair ----------
        accs = {}
        pend = []            # [(pt, qc, [egA, egB]), ...]

        def flush_av():
            while pend:
                pt, qc, egs = pend.pop(0)
                acc = accs[qc]
                for t, kb in enumerate((2 * pt, 2 * pt + 1)):
                    eg = egs[t]
                    cc, j = kb // 4, kb % 4
                    st, sp = (kb == 0), (kb == N_KB - 1)
                    nc.tensor.matmul(acc[0:32, :], Vt[cc][:, j, 0:32],
                                     eg[:, 0, :], start=st, stop=sp,
                                     skip_group_check=True,
                                     tile_position=(0, 0))
                    nc.tensor.matmul(acc[32:64, :], Vt[cc][:, j, 32:64],
                                     eg[:, 1, :], start=st, stop=sp,
                                     skip_group_check=True,
                                     tile_position=(0, 32))
                    nc.tensor.matmul(acc[64:65, :], Vt[cc][:, j, 64:65],
                                     eg[:, 0, :], start=st, stop=sp,
                                     skip_group_check=True,
                                     tile_position=(0, 64))
                    nc.tensor.matmul(acc[96:97, :], Vt[cc][:, j, 64:65],
                                     eg[:, 1, :], start=st, stop=sp,
                                     skip_group_check=True,
                                     tile_position=(0, 96))

        def attn_pair(pt, qc, n_pump=7):
            # four QK matmuls (row strips 0-3) back-to-back -> one PE window
            lgs = []
            for kb in (2 * pt, 2 * pt + 1):
                cc, j = kb // 4, kb % 4
                lg = qk_ps.tile([128, 2, 512], F32, tag="qk",
                                name=f"lg{qc}_{kb}")
                for hl in range(2):
                    i = 2 * (kb % 2) + hl
                    nc.tensor.matmul(
                        lg[:, hl, :],
                        kT[cc][32 * i:32 * i + 32, j * 128:(j + 1) * 128],
                        qTt[qc][32 * i:32 * i + 32, :],
                        start=True, stop=True, skip_group_check=True,
                        tile_position=(32 * i, 0))
                lgs.append(lg)
            egs = []
            for lg, kb in zip(lgs, (2 * pt, 2 * pt + 1)):
                eg = epool.tile([128, 2, 512], BF16, tag="eg",
                                name=f"eg{qc}_{kb}")
                nc.scalar.activation(eg, lg, AF.Exp)
                egs.append(eg)
            if pend:
                flush_av()
            pend.append((pt, qc, egs))
            pump(n_pump)

        def norm(qc):
            # 1/denominator via ACT: exp(-ln(d)); rb broadcasts it to 32 rows
            acc = accs.pop(qc)
            qsl = slice(qc * 512, (qc + 1) * 512)
            for hl in range(2):
                lnd = small.tile([1, 512], F32, tag="lnd", name=f"ln{qc}_{hl}")
                nc.scalar.activation(lnd, acc[64 + 32 * hl:65 + 32 * hl, :],
                                     AF.Ln)
                r = small.tile([1, 512], F32R, tag="recip", name=f"r{qc}_{hl}")
                nc.scalar.activation(r, lnd, AF.Exp, scale=-1.0)
                rb = aux_ps.tile([128, 512], F32, tag="aux", name=f"rb{qc}_{hl}")
                nc.tensor.matmul(rb[0:32, :], ones32, r, start=True, stop=True,
                                 skip_group_check=True)
                rbs = small.tile([32, 512], F32, tag="rbs", name=f"rbs{qc}_{hl}")
                nc.vector.tensor_copy(rbs, rb[0:32, :])
                nc.vector.tensor_mul(o_nrm[:, hl, qsl],
                                     acc[32 * hl:32 * hl + 32, :], rbs)

        # ---- scramble parts (shift-matmuls over a query sub-range) ---------
        o_j = o_nrm.rearrange("p h (m j) -> p h j m", j=8)   # [32, 2, 8, 256]

        def scramble_part(m0, m1):
            for icb in range(2):
                rp = aux_ps.tile([128, 512], F32, tag="aux",
                                 name=f"rp{icb}_{m0}")
                for hl in range(2):
                    for jm in range(4):
                        j = 4 * icb + jm
                        nc.tensor.matmul(
                            rp[:, 256 * hl + m0:256 * hl + m1],
                            identsh[:, jm, :], o_j[:, hl, j, m0:m1],
                            start=(jm == 0), stop=(jm == 3),
                            skip_group_check=True)
                for hl in range(2):
                    nc.vector.tensor_copy(
                        rowsT[:, icb, 256 * hl + m0:256 * hl + m1],
                        rp[:, 256 * hl + m0:256 * hl + m1])

        # ---- schedule -------------------------------------------------------
        for f in qconv_ops(0):
            f()
        for f in kvconv_ops(0):
            f()
        accs[0] = av_ps.tile([128, 512], F32, tag="av", name="acc0")
        accs[1] = av_ps.tile([128, 512], F32, tag="av", name="acc1")
        attn_pair(0, 0)
        for f in qconv_ops(1):
            f()
        queue_conv(1, kvconv_ops(1))
        attn_pair(0, 1)
        attn_pair(1, 0)
        attn_pair(1, 1)
        for c in range(1, 8):
            drain(c)                       # kvconv(c) must be complete
            if c < 7:
                queue_conv(c + 1, kvconv_ops(c + 1))
            else:
                queue_conv(8, qconv_ops(2))
                queue_conv(9, qconv_ops(3))
            for pr in (2 * c, 2 * c + 1):
                attn_pair(pr, 0)
                attn_pair(pr, 1)
        drain(8)                           # finish qconv(2)
        flush_av()
        norm(0)
        norm(1)

        accs[2] = av_ps.tile([128, 512], F32, tag="av", name="acc2")
        accs[3] = av_ps.tile([128, 512], F32, tag="av", name="acc3")
        for pt in range(N_KB // 2):
            attn_pair(pt, 2)
            if pt == 2:
                scramble_part(0, 128)      # qc0+qc1 queries
        drain(9)                           # finish qconv(3)
        flush_av()
        norm(2)
        for pt in range(N_KB // 2):
            attn_pair(pt, 3)
            if pt == 2:
                scramble_part(128, 192)    # qc2 queries
        flush_av()
        norm(3)
        scramble_part(192, 256)            # qc3 queries

        # ---- output projection ---------------------------------------------
        for ocb in range(2):
            po = aux_ps.tile([128, 512], F32, tag="aux", name=f"po{ocb}")
            for icb in range(2):
                nc.tensor.matmul(po, wpt_sb[:, icb, 128 * ocb:128 * ocb + 128],
                                 rowsT[:, icb, :], start=(icb == 0),
                                 stop=(icb == 1), skip_group_check=True)
            ob = small.tile([128, 512], F32, tag="outsb", name=f"ob{ocb}")
            nc.vector.tensor_scalar_add(ob, po, bp_sb[:, ocb:ocb + 1])
            nc.sync.dma_start(out=out_d[128 * ocb:128 * ocb + 128, :], in_=ob)


# --------------------------------------------------------------------------
# host side
# --------------------------------------------------------------------------

def host_prep(x, attn_kv, qw, qb, kvw, kvb, Wq, bq, Wk, bk, Wv, bv, Wp, bp):
    import ml_dtypes
    f = np.float32
    bf = ml_dtypes.bfloat16
    x = np.asarray(x, f)[0]          # [256, 64, 64]
    akv = np.asarray(attn_kv, f)[0]
    Wqc = np.einsum("jc,ciyx->jiyx", np.asarray(Wq, f), np.asarray(qw, f)) * SCALE
    Wkc = np.einsum("jc,ciyx->jiyx", np.asarray(Wk, f), np.asarray(kvw, f))
    Wvc = np.einsum("jc,ciyx->jiyx", np.asarray(Wv, f), np.asarray(kvw, f))
    bqc = (np.asarray(Wq, f) @ np.asarray(qb, f) + np.asarray(bq, f)) * SCALE
    bkc = np.asarray(Wk, f) @ np.asarray(kvb, f) + np.asarray(bk, f)
    bvc = np.asarray(Wv, f) @ np.asarray(kvb, f) + np.asarray(bv, f)

    akv_p = np.zeros((DIM, 66, 66), f)
    akv_p[:, 1:65, 1:65] = akv
    x_p = np.zeros((DIM, 66, 66), f)
    x_p[:, 1:65, 1:65] = x

    per_pair = []
    for p in range(4):
        wkv = np.zeros((128, DIM, 3, 3), f)
        bkv = np.zeros((128,), f)
        wq_ = np.zeros((64, DIM, 3, 3), f)
        bq_ = np.zeros((64,), f)
        for hl in range(2):
            h = 2 * p + hl
            wkv[32 * hl:32 * hl + QK_DIM] = Wkc[QK_DIM * h:QK_DIM * (h + 1)]
            bkv[32 * hl:32 * hl + QK_DIM] = bkc[QK_DIM * h:QK_DIM * (h + 1)]
            wkv[64 + 32 * hl:64 + 32 * (hl + 1)] = Wvc[HEAD_DIM * h:HEAD_DIM * (h + 1)]
            bkv[64 + 32 * hl:64 + 32 * (hl + 1)] = bvc[HEAD_DIM * h:HEAD_DIM * (h + 1)]
            wq_[32 * hl:32 * hl + QK_DIM] = Wqc[QK_DIM * h:QK_DIM * (h + 1)]
            bq_[32 * hl:32 * hl + QK_DIM] = bqc[QK_DIM * h:QK_DIM * (h + 1)]
        per_pair.append((
            np.ascontiguousarray(wkv.transpose(1, 2, 3, 0).reshape(DIM, 9, 128)).astype(bf),
            bkv.reshape(128, 1),
            np.ascontiguousarray(wq_.transpose(1, 2, 3, 0).reshape(DIM, 9, 64)).astype(bf),
            bq_.reshape(64, 1),
        ))

    identsh = np.zeros((32, 4, 128), bf)
    for jm in range(4):
        for dd in range(32):
            identsh[dd, jm, 32 * jm + dd] = 1.0
    wpt = np.ascontiguousarray(np.asarray(Wp, f).T).astype(bf)
    bp_a = np.ascontiguousarray(np.asarray(bp, f).reshape(2, 128).T)

    akv_b = akv_p.reshape(DIM, -1).astype(bf)
    in_maps = []
    for c in range(N_CORES):
        p, s = c // 2, c % 2
        wkv_h, bkv_h, wq_h, bq_h = per_pair[p]
        in_maps.append({
            "akv": akv_b,
            "xs": np.ascontiguousarray(
                x_p[:, 32 * s:32 * s + 34, :]).reshape(DIM, -1).astype(bf),
            "wkv": wkv_h, "bkv": bkv_h, "wq": wq_h, "bq": bq_h,
            "ones32": np.ones((1, 32), f),
            "ident64": np.eye(64, dtype=f).astype(bf), "identsh": identsh,
            "wpt": wpt, "bp": bp_a,
        })
    return in_maps


def gather(results):
    full = np.empty((DIM, N), np.float32)
    for c in range(N_CORES):
        p, s = c // 2, c % 2
        dev = results[c]["out"]
        for hl in range(2):
            h = 2 * p + hl
            full[:, 512 * h + 256 * s: 512 * h + 256 * s + 256] = \
                dev[:, 256 * hl:256 * hl + 256]
    return full.reshape(1, DIM, H, W)


def kernel(x, attn_kv, qw, qb, kvw, kvb, Wq, bq, Wk, bk, Wv, bv, Wp, bp):
    nc = build_nc()
    in_maps = host_prep(x, attn_kv, qw, qb, kvw, kvb, Wq, bq, Wk, bk, Wv, bv,
                        Wp, bp)
    res = bass_utils.run_bass_kernel_spmd(nc, in_maps,
                                          core_ids=list(range(N_CORES)),
                                          trace=False)
    return gather(res.results).astype(np.float32)
